# revision 20
# baseline (speedup 1.0000x reference)
"""AttentionPooling (segment_reduce) on 8 TRN2 NeuronCores.

Math: pooled[s,:] = sum_{i: batch[i]=s} exp(score_i) * x[i,:] / sum_j exp(score_j)
with score = x @ W (+ b, which cancels in the softmax).  Scores ~ N(0,1) so
exp() is numerically safe without a max pass; normalization is one global
scalar (AllReduce), applied at the end.

Strategy (segment sharding):
  - Core c owns segments [c*512, (c+1)*512) = 4 blocks of 128 segments.
  - batch_index is sorted, so each 128-segment block's nodes are one
    contiguous row range; host routes each block's rows to its owning core,
    padded to a uniform nbsub subtiles of 128 nodes (SPMD: one graph).
  - Host also precomputes the raw 0/1 one-hot (pure index data) in bf16.
  - Per 128-node subtile on device:
      score   = scalar_tensor_tensor(x_sub * W_bc, accum)   (DVE)
      escore  = exp(scores)                                 (ACT, 8/op)
      ohw     = oh_raw * escore                             (DVE ts_mul)
      psum   += ohw.T @ x_sub                               (PE scatter matmul)
    Pad rows have lidx -1 => one-hot row of zeros => no pooled contribution;
    their exp(0)=1 denominator excess is removed via host pad counts.
  - One AllReduce of the scalar denominator; divide; each core outputs its
    [512, 256] shard; host concatenates.
"""

import sys

import numpy as np

for _p in ("/opt/trn_rl_repo",):
    if _p not in sys.path:
        sys.path.insert(0, _p)

N_SEG = 4096
D = 256
N_CORES = 8
SEG_BLOCK = 128          # segments per PSUM block (= PE stationary free dim)
BLOCKS_PER_CORE = 4      # 512 segments per core
SUPER = 2048             # nodes per DMA super-tile (16 subtiles)


def _pack_inputs(x, idx, w, bias):
    """Route each 128-segment block's (contiguous) rows to its owning core."""
    import ml_dtypes

    bf16 = ml_dtypes.bfloat16
    bounds = np.searchsorted(idx, np.arange(0, N_SEG + 1, SEG_BLOCK)).astype(np.int64)
    counts = np.diff(bounds)
    nbsub = int(np.ceil(max(int(counts.max()), 1) / 128))   # subtiles per block
    s_sub = BLOCKS_PER_CORE * nbsub                          # subtiles per core
    t_nodes = int(np.ceil(s_sub * 128 / SUPER)) * SUPER      # padded nodes per core
    nst = t_nodes // SUPER

    wbc = np.tile(np.asarray(w, np.float32).reshape(D), (128, 1)).astype(bf16)
    ones = np.ones((128, 1), np.float32)

    # DMA layout permutation: SBUF super-tile st, partition p, chunk k reads
    # flat row st*1024 + p*8 + k, which must hold logical node (st*8+k)*128+p
    # (so that subtile j = st*8+k covers logical rows [j*128, (j+1)*128)).
    i = np.arange(t_nodes)
    perm = ((i // SUPER) * 16 + (i % 16)) * 128 + (i % SUPER) // 16

    seg_ar = np.arange(SEG_BLOCK, dtype=np.float32)
    in_maps = []
    for c in range(N_CORES):
        xl = np.zeros((t_nodes, D), np.float32)
        li = np.full(t_nodes, -1.0, np.float32)
        for blk in range(BLOCKS_PER_CORE):
            g = c * BLOCKS_PER_CORE + blk
            s, e = int(bounds[g]), int(bounds[g + 1])
            cnt = e - s
            off = blk * nbsub * 128
            xl[off : off + cnt] = x[s:e]
            li[off : off + cnt] = (idx[s:e] - g * SEG_BLOCK).astype(np.float32)
        # one-hot rows in SBUF layout: [p, j*128 + seg] <- logical node j*128+p
        licols = np.ascontiguousarray(li[: s_sub * 128].reshape(s_sub, 128).T)
        oh = (licols[:, :, None] == seg_ar[None, None, :]).astype(bf16)
        ohp = np.zeros((128, nst * 16, SEG_BLOCK), bf16)
        ohp[:, :s_sub] = oh
        # pads: lidx=-1 -> all-zero one-hot row; exp(0)=1 pollutes only the
        # denominator -> subtract per-partition pad counts.
        pad_per_part = (licols < 0).sum(axis=1).astype(np.float32)
        in_maps.append(
            {
                "x": np.ascontiguousarray(xl[perm]).astype(bf16),
                "oh": np.ascontiguousarray(
                    ohp.reshape(128, nst, 16, SEG_BLOCK)
                    .transpose(1, 0, 2, 3)
                    .reshape(nst * 128, 16 * SEG_BLOCK)
                ),
                "padadj": -pad_per_part.reshape(128, 1),
                "wbc": wbc,
                "ones": ones,
            }
        )
    return in_maps, nbsub, t_nodes


def _build(nbsub, t_nodes):
    from concourse import bacc, mybir, tile

    nc = bacc.Bacc("TRN2", target_bir_lowering=False, debug=False,
                   num_devices=N_CORES)
    f32 = mybir.dt.float32
    bf16 = mybir.dt.bfloat16
    s_sub = BLOCKS_PER_CORE * nbsub
    nst = t_nodes // SUPER

    x_ext = nc.dram_tensor("x", [t_nodes, D], bf16, kind="ExternalInput")
    oh_ext = nc.dram_tensor(
        "oh", [nst * 128, 16 * SEG_BLOCK], bf16, kind="ExternalInput"
    )
    padadj_ext = nc.dram_tensor("padadj", [128, 1], f32, kind="ExternalInput")
    wbc_ext = nc.dram_tensor("wbc", [128, D], bf16, kind="ExternalInput")
    ones_ext = nc.dram_tensor("ones", [128, 1], f32, kind="ExternalInput")
    out_ext = nc.dram_tensor(
        "out", [BLOCKS_PER_CORE * SEG_BLOCK, D], f32, kind="ExternalOutput"
    )

    x_src = x_ext.ap().rearrange("(s p k) d -> s p (k d)", p=128, k=16)
    oh_src = oh_ext.ap().rearrange("(s p) c -> s p c", p=128)
    out_dst = out_ext.ap().rearrange("(b p) d -> b p d", p=SEG_BLOCK)

    with tile.TileContext(nc) as tc:
        with (
            tc.tile_pool(name="const", bufs=1) as constp,
            tc.tile_pool(name="xin", bufs=8) as xp,
            tc.tile_pool(name="ohin", bufs=8) as ohp,
            tc.tile_pool(name="scratch", bufs=6) as scrp,
            tc.tile_pool(name="small", bufs=9) as smp,
            tc.tile_pool(name="accs", bufs=1) as accp,
            tc.tile_pool(name="outp", bufs=1) as outp,
            tc.tile_pool(name="psum", bufs=2, space="PSUM") as psp,
            tc.tile_pool(name="psd", bufs=1, space="PSUM") as psd,
            tc.tile_pool(name="dram", bufs=1, space="DRAM") as dramp,
        ):
            wbc = constp.tile([128, D], bf16, name="wbc_sb")
            nc.scalar.dma_start(wbc[:], wbc_ext.ap())
            padadj = constp.tile([128, 1], f32, name="padadj_sb")
            nc.scalar.dma_start(padadj[:], padadj_ext.ap())
            ones = constp.tile([128, 1], f32, name="ones_sb")
            nc.scalar.dma_start(ones[:], ones_ext.ap())

            den_in = dramp.tile([1, 1], f32, name="den_in")
            den_out = dramp.tile([1, 1], f32, name="den_out", addr_space="Shared")
            warm_in = dramp.tile([1, 1], f32, name="warm_in")
            warm_out = dramp.tile([1, 1], f32, name="warm_out", addr_space="Shared")
            warm_sb = constp.tile([1, 1], f32, name="warm_sb")
            nc.vector.memset(warm_sb[:], 0.0)
            nc.gpsimd.dma_start(warm_in[:], warm_sb[:])
            # dummy collective: wakes the collective firmware early so the real
            # AllReduce at the tail doesn't pay the ~10us cold-start
            nc.gpsimd.collective_compute(
                "AllReduce",
                mybir.AluOpType.add,
                replica_groups=[list(range(N_CORES))],
                ins=[warm_in.opt()],
                outs=[warm_out.opt()],
            )
            essum = accp.tile([128, nst], f32, name="essum")
            pooled_all = outp.tile([128, BLOCKS_PER_CORE * D], f32,
                                   name="pooled_all")

            xt_tiles = {}
            oh_tiles = {}
            es_tiles = {}
            ps = None
            LEAD = 2  # chunks the score/exp stream runs ahead of the matmuls

            def emit_front(st):
                xt = xp.tile([128, SUPER * 2], bf16, tag="xt", name="xt")
                nc.sync.dma_start(xt[:], x_src[st])
                xt_tiles[st] = xt
                oht = ohp.tile([128, 16 * SEG_BLOCK], bf16, tag="oht", name="oht")
                nc.gpsimd.dma_start(oht[:], oh_src[st])
                oh_tiles[st] = oht
                sc8 = smp.tile([128, 16], f32, tag="sc8", name="sc8")
                for kk in range(16):
                    tmp = scrp.tile([128, D], bf16, tag="tmp", name="tmp")
                    nc.vector.scalar_tensor_tensor(
                        out=tmp[:],
                        in0=xt[:, kk * D : (kk + 1) * D],
                        scalar=0.0,
                        in1=wbc[:],
                        op0=mybir.AluOpType.bypass,
                        op1=mybir.AluOpType.mult,
                        accum_out=sc8[:, kk : kk + 1],
                    )
                es8 = smp.tile([128, 16], f32, tag="es8", name="es8")
                nc.scalar.activation(
                    out=es8[:],
                    in_=sc8[:],
                    func=mybir.ActivationFunctionType.Exp,
                    accum_out=essum[:, st : st + 1],
                )
                es_tiles[st] = es8

            def emit_back(st):
                nonlocal ps
                xt = xt_tiles[st]
                oht = oh_tiles[st]
                es8 = es_tiles[st]
                for k in range(16):
                    j = st * 16 + k
                    if j >= s_sub:
                        break
                    blk, jb = j // nbsub, j % nbsub
                    if jb == 0:
                        ps = psp.tile([128, D], f32, tag="ps", name="ps")
                    ohw = scrp.tile([128, SEG_BLOCK], bf16, tag="ohw", name="ohw")
                    if k % 8 < 5:
                        # most one-hot scalings on the otherwise idle ACT engine
                        nc.scalar.activation(
                            out=ohw[:],
                            in_=oht[:, k * SEG_BLOCK : (k + 1) * SEG_BLOCK],
                            func=mybir.ActivationFunctionType.Copy,
                            scale=es8[:, k : k + 1],
                        )
                    else:
                        nc.vector.tensor_scalar(
                            out=ohw[:],
                            in0=oht[:, k * SEG_BLOCK : (k + 1) * SEG_BLOCK],
                            scalar1=es8[:, k : k + 1],
                            scalar2=None,
                            op0=mybir.AluOpType.mult,
                        )
                    nc.tensor.matmul(
                        ps[:],
                        ohw[:],
                        xt[:, k * D : (k + 1) * D],
                        start=(jb == 0),
                        stop=(jb == nbsub - 1),
                    )
                    if jb == nbsub - 1:
                        nc.scalar.copy(
                            pooled_all[:, blk * D : (blk + 1) * D], ps[:]
                        )
                xt_tiles.pop(st)
                oh_tiles.pop(st)
                es_tiles.pop(st)

            assert s_sub % 16 == 0 or s_sub <= nst * 16
            n_chunks = (s_sub + 15) // 16
            for st in range(n_chunks + LEAD):
                if st < n_chunks:
                    emit_front(st)
                if st == n_chunks - 1:
                    # all exps emitted -> emit the denominator AllReduce now so
                    # it overlaps the trailing LEAD chunks of matmul work
                    acc = smp.tile([128, 1], f32, name="acc")
                    nc.vector.tensor_reduce(
                        acc[:], essum[:], axis=mybir.AxisListType.X,
                        op=mybir.AluOpType.add,
                    )
                    nc.vector.tensor_add(acc[:], acc[:], padadj[:])
                    den_ps = psd.tile([1, 1], f32, name="den_ps")
                    nc.tensor.matmul(den_ps[:], acc[:], ones[:], start=True,
                                     stop=True)
                    den_sb = smp.tile([1, 1], f32, name="den_sb")
                    nc.scalar.copy(den_sb[:], den_ps[:])
                    nc.sync.dma_start(den_in[:], den_sb[:])
                    nc.gpsimd.collective_compute(
                        "AllReduce",
                        mybir.AluOpType.add,
                        replica_groups=[list(range(N_CORES))],
                        ins=[den_in.opt()],
                        outs=[den_out.opt()],
                    )
                if st >= LEAD:
                    emit_back(st - LEAD)

            den_bc = smp.tile([128, 1], f32, name="den_bc")
            nc.sync.dma_start(den_bc[:], den_out[:].broadcast_to((128, 1)))
            rbc = smp.tile([128, 1], f32, name="rbc")
            nc.vector.reciprocal(rbc[:], den_bc[:])

            fin = outp.tile([128, BLOCKS_PER_CORE * D], f32, name="fin")
            nc.vector.tensor_scalar(
                out=fin[:], in0=pooled_all[:], scalar1=rbc[:, 0:1],
                scalar2=None, op0=mybir.AluOpType.mult,
            )
            nc.sync.dma_start(
                out_ext.ap().rearrange("(b p) d -> p b d", p=SEG_BLOCK), fin[:]
            )

    nc.compile()
    return nc


def _run(inputs, trace=False):
    from concourse import bass_utils

    x = np.ascontiguousarray(np.asarray(inputs["node_features"], np.float32))
    idx = np.asarray(inputs["batch_index"]).astype(np.int64)
    w = np.asarray(inputs["W"], np.float32)
    bias = float(np.asarray(inputs["b"], np.float32).reshape(-1)[0])

    in_maps, nbsub, t_nodes = _pack_inputs(x, idx, w, bias)
    nc = _build(nbsub, t_nodes)
    res = bass_utils.run_bass_kernel_spmd(
        nc, in_maps, core_ids=list(range(N_CORES)), trace=trace
    )
    out = np.concatenate([res.results[c]["out"] for c in range(N_CORES)], axis=0)
    return out, res


def kernel(node_features, batch_index, num_segments=N_SEG, W=None, b=None):
    out, _ = _run(
        {
            "node_features": node_features,
            "batch_index": batch_index,
            "num_segments": num_segments,
            "W": W,
            "b": b,
        }
    )
    return out
